# Initial kernel scaffold
#
"""Trainium2 Bass kernel for nn_CyclicShuffle: grouped 1x1 conv with activation/weight
quantization, BN (inference) + ReLU + residual.

Strategy: data-parallel over batch (64 batches -> 8 per core). Per core:
  - Quantized activations are exact integers 0..15; quantized weights are exact odd
    integers -15..15, so the grouped conv runs exactly on the PE in bf16 with fp32
    PSUM accumulation. The 1/(15*15) factor is folded into the BN scale.
  - Quant pipeline: ACT fma (scale/bias) -> DVE magic-round+upper-clip ->
    DVE lower-clip+unshift with bf16 cast.
  - Per output block: ACT Relu(psum*scale_c + bias_c) with per-partition scale/bias,
    then DVE tensor_tensor add of the residual (in-place into the x tile), DMA out.
Self-contained: shapes hardcoded, no sibling imports.
"""

import numpy as np

B, C, HW = 64, 1024, 784          # x: [64, 1024, 28, 28] fp32
G, CG = 4, 256
N_CORES = 8
BPC = B // N_CORES                 # batches per core
NBLK = C // 128                    # 8 channel blocks of 128
NHALF = HW // 2                    # 392 (psum bank limit is 512 fp32)
MAGIC = float(2.0 ** 23)

_COMPILED = None


def _build_program(s_a, b_a, s_w, neg_lw, eps):
    """Build the SPMD Bass/Tile program. Scalar quant params are baked as immediates."""
    from contextlib import ExitStack
    from concourse import bacc, bass, tile, masks, mybir

    f32 = mybir.dt.float32
    bf16 = mybir.dt.bfloat16
    AF = mybir.ActivationFunctionType
    OP = mybir.AluOpType

    nc = bacc.Bacc("TRN2", target_bir_lowering=False, debug=False)

    x_d = nc.dram_tensor("x", [BPC, C, HW], f32, kind="ExternalInput")
    w_d = nc.dram_tensor("w", [C, CG], f32, kind="ExternalInput")
    gamma_d = nc.dram_tensor("gamma", [C], f32, kind="ExternalInput")
    beta_d = nc.dram_tensor("beta", [C], f32, kind="ExternalInput")
    mean_d = nc.dram_tensor("mean", [C], f32, kind="ExternalInput")
    var_d = nc.dram_tensor("var", [C], f32, kind="ExternalInput")
    y_d = nc.dram_tensor("y", [BPC, C, HW], f32, kind="ExternalOutput")

    with tile.TileContext(nc) as tc, ExitStack() as ctx:
        const = ctx.enter_context(tc.tile_pool(name="const", bufs=1))
        wpool = ctx.enter_context(tc.tile_pool(name="wprep", bufs=1))
        xpool = ctx.enter_context(tc.tile_pool(name="x", bufs=18))
        tpool = ctx.enter_context(tc.tile_pool(name="t", bufs=4))
        apool = ctx.enter_context(tc.tile_pool(name="a", bufs=12))
        rpool = ctx.enter_context(tc.tile_pool(name="r", bufs=6))
        pspool = ctx.enter_context(tc.tile_pool(name="ps", bufs=6, space="PSUM"))
        wtps = ctx.enter_context(tc.tile_pool(name="wtps", bufs=2, space="PSUM"))

        # ---------- one-time: weight quantization + transpose ----------
        ident = const.tile([128, 128], bf16)
        masks.make_identity(nc, ident[:])

        wq = wpool.tile([128, NBLK * CG], f32)     # wq[p, blk*256+k] = w[blk*128+p, k]
        for blk in range(NBLK):
            nc.sync.dma_start(out=wq[:, blk * CG:(blk + 1) * CG],
                              in_=w_d[blk * 128:(blk + 1) * 128, :])
        # u = (w - lW) * s_w  (two roundings, matches ref div-then-mul up to 1ulp)
        nc.vector.tensor_scalar(out=wq[:], in0=wq[:], scalar1=neg_lw, scalar2=s_w,
                                op0=OP.add, op1=OP.mult)
        # shifted round + upper clip
        nc.vector.tensor_scalar(out=wq[:], in0=wq[:], scalar1=MAGIC,
                                scalar2=MAGIC + 15.0, op0=OP.add, op1=OP.min)
        # lower clip + (2q - 15):  2*v - (2M+15)
        wint = wpool.tile([128, NBLK * CG], bf16)
        nc.vector.tensor_scalar(out=wq[:], in0=wq[:], scalar1=MAGIC, scalar2=2.0,
                                op0=OP.max, op1=OP.mult)
        nc.vector.tensor_scalar(out=wint[:], in0=wq[:], scalar1=2.0 * MAGIC + 15.0,
                                scalar2=None, op0=OP.subtract)
        # transpose the 16 [128,128] chunks: WT[:, (j*2+kc)*128+m] = wint[m, j*256+kc*128+:]
        wt = const.tile([128, 16 * 128], bf16)
        for j in range(NBLK):
            for kc in range(2):
                pst = wtps.tile([128, 128], f32)
                nc.tensor.transpose(pst[:], wint[:, j * CG + kc * 128: j * CG + (kc + 1) * 128],
                                    ident[:])
                nc.vector.tensor_copy(wt[:, (j * 2 + kc) * 128:(j * 2 + kc + 1) * 128], pst[:])

        # ---------- one-time: BN fold ----------
        # S[p, j] = gamma/(225*sqrt(var+eps)) for channel c = j*128+p ; Bc = beta - mean*inv
        def load_param(dram):
            t = const.tile([128, NBLK], f32)
            nc.sync.dma_start(out=t[:], in_=dram.ap().rearrange("(a p) -> p a", p=128))
            return t

        g_t, b_t, m_t, v_t = (load_param(d) for d in (gamma_d, beta_d, mean_d, var_d))
        sq = const.tile([128, NBLK], f32)
        nc.scalar.activation(sq[:], v_t[:], AF.Sqrt, scale=50625.0, bias=float(50625.0 * eps))
        rec = const.tile([128, NBLK], f32)
        nc.vector.reciprocal(rec[:], sq[:])
        s_t = const.tile([128, NBLK], f32)
        nc.vector.tensor_tensor(out=s_t[:], in0=g_t[:], in1=rec[:], op=OP.mult)
        bc_t = const.tile([128, NBLK], f32)
        nc.vector.scalar_tensor_tensor(out=bc_t[:], in0=m_t[:], scalar=-225.0, in1=s_t[:],
                                       op0=OP.mult, op1=OP.mult)
        nc.vector.tensor_tensor(out=bc_t[:], in0=bc_t[:], in1=b_t[:], op=OP.add)

        # ---------- main loop ----------
        for b in range(BPC):
            xt = []
            at = []
            for j in range(NBLK):
                xj = xpool.tile([128, HW], f32, tag="x")
                nc.sync.dma_start(out=xj[:], in_=x_d[b, j * 128:(j + 1) * 128, :])
                xt.append(xj)
            for j in range(NBLK):
                tj = tpool.tile([128, HW], f32, tag="t")
                # t = s_a*x + b_a   (ACT free affine, Identity)
                nc.scalar.activation(tj[:], xt[j][:], AF.Identity, scale=float(s_a), bias=float(b_a))
                # shifted round + upper clip
                nc.vector.tensor_scalar(out=tj[:], in0=tj[:], scalar1=MAGIC,
                                        scalar2=MAGIC + 15.0, op0=OP.add, op1=OP.min)
                aj = apool.tile([128, HW], bf16, tag="a")
                # lower clip + unshift, cast bf16 (exact integers 0..15)
                nc.vector.tensor_scalar(out=aj[:], in0=tj[:], scalar1=MAGIC,
                                        scalar2=-MAGIC, op0=OP.max, op1=OP.add)
                at.append(aj)
            for g in range(G):
                src0 = (2 * g + 2) % NBLK   # first k-block of source group (g+1)%4
                for oc in range(2):
                    j = 2 * g + oc
                    ps = [pspool.tile([128, NHALF], f32, tag="ps") for _ in range(2)]
                    for kc in range(2):
                        lhsT = wt[:, (j * 2 + kc) * 128:(j * 2 + kc + 1) * 128]
                        rhs_t = at[src0 + kc]
                        for half in range(2):
                            nc.tensor.matmul(
                                ps[half][:], lhsT, rhs_t[:, half * NHALF:(half + 1) * NHALF],
                                start=(kc == 0), stop=(kc == 1))
                    for half in range(2):
                        rt = rpool.tile([128, NHALF], f32, tag="r")
                        nc.scalar.activation(rt[:], ps[half][:], AF.Relu,
                                             scale=s_t[:, j:j + 1], bias=bc_t[:, j:j + 1])
                        sl = xt[j][:, half * NHALF:(half + 1) * NHALF]
                        nc.vector.tensor_tensor(out=sl, in0=rt[:], in1=sl, op=OP.add)
                    nc.sync.dma_start(out=y_d[b, j * 128:(j + 1) * 128, :], in_=xt[j][:])

    nc.compile()
    return nc


def kernel(x, weight, lW, uW, lA, uA, gamma, beta, running_mean, running_var):
    global _COMPILED
    from concourse.bass_utils import run_bass_kernel_spmd

    x = np.ascontiguousarray(np.asarray(x, dtype=np.float32)).reshape(B, C, HW)
    weight = np.ascontiguousarray(np.asarray(weight, dtype=np.float32))
    lW = np.float32(lW); uW = np.float32(uW); lA = np.float32(lA); uA = np.float32(uA)
    gamma = np.ascontiguousarray(np.asarray(gamma, dtype=np.float32))
    beta = np.ascontiguousarray(np.asarray(beta, dtype=np.float32))
    mean = np.ascontiguousarray(np.asarray(running_mean, dtype=np.float32))
    var = np.ascontiguousarray(np.asarray(running_var, dtype=np.float32))

    s_a = np.float32(15.0) / (uA - lA)
    b_a = -lA * s_a
    s_w = np.float32(15.0) / np.float32(uW - lW)

    key = (float(s_a), float(b_a), float(s_w), float(-lW))
    if _COMPILED is None or _COMPILED[0] != key:
        nc = _build_program(float(s_a), float(b_a), float(s_w), float(-lW), 1e-5)
        _COMPILED = (key, nc)
    nc = _COMPILED[1]

    in_maps = []
    for c in range(N_CORES):
        in_maps.append({
            "x": x[c * BPC:(c + 1) * BPC],
            "w": weight,
            "gamma": gamma, "beta": beta, "mean": mean, "var": var,
        })
    res = run_bass_kernel_spmd(nc, in_maps, list(range(N_CORES)))
    out = np.concatenate([res.results[c]["y"] for c in range(N_CORES)], axis=0)
    return out.reshape(B, C, 28, 28).astype(np.float32)


# revision 10
# speedup vs baseline: 2.4633x; 2.4633x over previous
"""Trainium2 Bass kernel for nn_CyclicShuffle: grouped 1x1 conv with activation/weight
quantization, BN (inference) + ReLU + residual.

Strategy: data-parallel over batch (64 batches -> 8 per core). Per core:
  - Quantized activations are exact integers 0..15; quantized weights are exact odd
    integers -15..15, so the grouped conv runs exactly on the PE in bf16 with fp32
    PSUM accumulation. The 1/(15*15) factor is folded into the BN scale.
  - Quant pipeline: ACT fma (scale/bias) -> DVE magic-round+upper-clip ->
    DVE lower-clip+unshift with bf16 cast.
  - Per output block: ACT Relu(psum*scale_c + bias_c) with per-partition scale/bias,
    then DVE tensor_tensor add of the residual (in-place into the x tile), DMA out.
Self-contained: shapes hardcoded, no sibling imports.
"""

import numpy as np

B, C, HW = 64, 1024, 784          # x: [64, 1024, 28, 28] fp32
G, CG = 4, 256
N_CORES = 8
BPC = B // N_CORES                 # batches per core
NBLK = C // 128                    # 8 channel blocks of 128
NHALF = HW // 2                    # 392 (psum bank limit is 512 fp32)
MAGIC = float(2.0 ** 23)

_COMPILED = None


def _build_program(s_a, b_a, s_w, neg_lw, eps, repeats=1):
    """Build the SPMD Bass/Tile program. Scalar quant params are baked as immediates.
    repeats>1 duplicates the main loop (same I/O) for slope-based timing."""
    from contextlib import ExitStack
    from concourse import bacc, bass, tile, masks, mybir

    f32 = mybir.dt.float32
    bf16 = mybir.dt.bfloat16
    AF = mybir.ActivationFunctionType
    OP = mybir.AluOpType

    nc = bacc.Bacc("TRN2", target_bir_lowering=False, debug=False)

    x_d = nc.dram_tensor("x", [BPC, C, HW], f32, kind="ExternalInput")
    w_d = nc.dram_tensor("w", [C, CG], f32, kind="ExternalInput")
    gamma_d = nc.dram_tensor("gamma", [C], f32, kind="ExternalInput")
    beta_d = nc.dram_tensor("beta", [C], f32, kind="ExternalInput")
    mean_d = nc.dram_tensor("mean", [C], f32, kind="ExternalInput")
    var_d = nc.dram_tensor("var", [C], f32, kind="ExternalInput")
    y_d = nc.dram_tensor("y", [BPC, C, HW], f32, kind="ExternalOutput")

    with tile.TileContext(nc) as tc, ExitStack() as ctx:
        const = ctx.enter_context(tc.tile_pool(name="const", bufs=1))
        wpool = ctx.enter_context(tc.tile_pool(name="wprep", bufs=1))
        xpool = ctx.enter_context(tc.tile_pool(name="x", bufs=18))
        tpool = ctx.enter_context(tc.tile_pool(name="t", bufs=4))
        apool = ctx.enter_context(tc.tile_pool(name="a", bufs=12))
        rpool = ctx.enter_context(tc.tile_pool(name="r", bufs=6))
        pspool = ctx.enter_context(tc.tile_pool(name="ps", bufs=6, space="PSUM"))
        wtps = ctx.enter_context(tc.tile_pool(name="wtps", bufs=2, space="PSUM"))

        # ---------- one-time: weight quantization + transpose ----------
        ident = const.tile([128, 128], bf16)
        masks.make_identity(nc, ident[:])

        wq = wpool.tile([128, NBLK * CG], f32)     # wq[p, blk*256+k] = w[blk*128+p, k]
        for blk in range(NBLK):
            nc.gpsimd.dma_start(out=wq[:, blk * CG:(blk + 1) * CG],
                                in_=w_d[blk * 128:(blk + 1) * 128, :])
        # u = (w - lW) * s_w  (two roundings, matches ref div-then-mul up to 1ulp)
        nc.vector.tensor_scalar(out=wq[:], in0=wq[:], scalar1=neg_lw, scalar2=s_w,
                                op0=OP.add, op1=OP.mult)
        # shifted round + upper clip
        nc.vector.tensor_scalar(out=wq[:], in0=wq[:], scalar1=MAGIC,
                                scalar2=MAGIC + 15.0, op0=OP.add, op1=OP.min)
        # lower clip + unshift -> q in 0..15 (2M+15 is NOT fp32-representable, so
        # unshift before the affine)
        wint = wpool.tile([128, NBLK * CG], bf16)
        nc.vector.tensor_scalar(out=wq[:], in0=wq[:], scalar1=MAGIC, scalar2=-MAGIC,
                                op0=OP.max, op1=OP.add)
        # w_int = 2q - 15 (odd integers -15..15, exact in bf16)
        nc.vector.tensor_scalar(out=wint[:], in0=wq[:], scalar1=2.0, scalar2=-15.0,
                                op0=OP.mult, op1=OP.add)
        # transpose the 16 [128,128] chunks: WT[:, (j*2+kc)*128+m] = wint[m, j*256+kc*128+:]
        wt = const.tile([128, 16 * 128], bf16)
        for j in range(NBLK):
            for kc in range(2):
                pst = wtps.tile([128, 128], bf16)
                nc.tensor.transpose(pst[:], wint[:, j * CG + kc * 128: j * CG + (kc + 1) * 128],
                                    ident[:])
                nc.vector.tensor_copy(wt[:, (j * 2 + kc) * 128:(j * 2 + kc + 1) * 128], pst[:])

        # ---------- one-time: BN fold ----------
        # S[p, j] = gamma/(225*sqrt(var+eps)) for channel c = j*128+p ; Bc = beta - mean*inv
        def load_param(dram, nm):
            t = const.tile([128, NBLK], f32, name=nm, tag=nm)
            nc.gpsimd.dma_start(out=t[:], in_=dram.ap().rearrange("(a p) -> p a", p=128))
            return t

        g_t = load_param(gamma_d, "g_t")
        b_t = load_param(beta_d, "b_t")
        m_t = load_param(mean_d, "m_t")
        v_t = load_param(var_d, "v_t")
        eps_t = const.tile([128, 1], f32)
        nc.gpsimd.memset(eps_t[:], float(50625.0 * eps))
        sq = const.tile([128, NBLK], f32)
        nc.scalar.activation(sq[:], v_t[:], AF.Sqrt, scale=50625.0, bias=eps_t[:])
        rec = const.tile([128, NBLK], f32)
        nc.vector.reciprocal(rec[:], sq[:])
        s_t = const.tile([128, NBLK], f32)
        nc.vector.tensor_tensor(out=s_t[:], in0=g_t[:], in1=rec[:], op=OP.mult)
        bc_t = const.tile([128, NBLK], f32)
        nc.vector.scalar_tensor_tensor(out=bc_t[:], in0=m_t[:], scalar=-225.0, in1=s_t[:],
                                       op0=OP.mult, op1=OP.mult)
        nc.vector.tensor_tensor(out=bc_t[:], in0=bc_t[:], in1=b_t[:], op=OP.add)

        # ---------- main loop ----------
        for b in [bb for _ in range(repeats) for bb in range(BPC)]:
            xt = []
            at = []
            for j in range(NBLK):
                xj = xpool.tile([128, HW], f32, tag="x")
                nc.sync.dma_start(out=xj[:], in_=x_d[b, j * 128:(j + 1) * 128, :])
                xt.append(xj)
            for j in range(NBLK):
                tj = tpool.tile([128, HW], f32, tag="t")
                # t = s_a*x + b_a   (ACT free affine; Copy allows float bias)
                nc.scalar.activation(tj[:], xt[j][:], AF.Copy, scale=float(s_a), bias=float(b_a))
                # shifted round + upper clip
                nc.vector.tensor_scalar(out=tj[:], in0=tj[:], scalar1=MAGIC,
                                        scalar2=MAGIC + 15.0, op0=OP.add, op1=OP.min)
                aj = apool.tile([128, HW], bf16, tag="a")
                # lower clip + unshift, cast bf16 (exact integers 0..15)
                nc.vector.tensor_scalar(out=aj[:], in0=tj[:], scalar1=MAGIC,
                                        scalar2=-MAGIC, op0=OP.max, op1=OP.add)
                at.append(aj)
            for g in range(G):
                src0 = (2 * g + 2) % NBLK   # first k-block of source group (g+1)%4
                for oc in range(2):
                    j = 2 * g + oc
                    ps = [pspool.tile([128, NHALF], f32, tag="ps", name=f"ps{b}_{j}_{h}")
                          for h in range(2)]
                    for kc in range(2):
                        lhsT = wt[:, (j * 2 + kc) * 128:(j * 2 + kc + 1) * 128]
                        rhs_t = at[src0 + kc]
                        for half in range(2):
                            nc.tensor.matmul(
                                ps[half][:], lhsT, rhs_t[:, half * NHALF:(half + 1) * NHALF],
                                start=(kc == 0), stop=(kc == 1))
                    for half in range(2):
                        rt = rpool.tile([128, NHALF], f32, tag="r")
                        nc.scalar.activation(rt[:], ps[half][:], AF.Relu,
                                             scale=s_t[:, j:j + 1], bias=bc_t[:, j:j + 1])
                        sl = xt[j][:, half * NHALF:(half + 1) * NHALF]
                        nc.vector.tensor_tensor(out=sl, in0=rt[:], in1=sl, op=OP.add)
                    nc.scalar.dma_start(out=y_d[b, j * 128:(j + 1) * 128, :], in_=xt[j][:])

    nc.compile()
    return nc


def kernel(x, weight, lW, uW, lA, uA, gamma, beta, running_mean, running_var):
    global _COMPILED
    from concourse.bass_utils import run_bass_kernel_spmd

    x = np.ascontiguousarray(np.asarray(x, dtype=np.float32)).reshape(B, C, HW)
    weight = np.ascontiguousarray(np.asarray(weight, dtype=np.float32))
    lW = np.float32(lW); uW = np.float32(uW); lA = np.float32(lA); uA = np.float32(uA)
    gamma = np.ascontiguousarray(np.asarray(gamma, dtype=np.float32))
    beta = np.ascontiguousarray(np.asarray(beta, dtype=np.float32))
    mean = np.ascontiguousarray(np.asarray(running_mean, dtype=np.float32))
    var = np.ascontiguousarray(np.asarray(running_var, dtype=np.float32))

    s_a = np.float32(15.0) / (uA - lA)
    b_a = -lA * s_a
    s_w = np.float32(15.0) / np.float32(uW - lW)

    key = (float(s_a), float(b_a), float(s_w), float(-lW))
    if _COMPILED is None or _COMPILED[0] != key:
        nc = _build_program(float(s_a), float(b_a), float(s_w), float(-lW), 1e-5)
        _COMPILED = (key, nc)
    nc = _COMPILED[1]

    in_maps = []
    for c in range(N_CORES):
        in_maps.append({
            "x": x[c * BPC:(c + 1) * BPC],
            "w": weight,
            "gamma": gamma, "beta": beta, "mean": mean, "var": var,
        })
    res = run_bass_kernel_spmd(nc, in_maps, list(range(N_CORES)))
    out = np.concatenate([res.results[c]["y"] for c in range(N_CORES)], axis=0)
    return out.reshape(B, C, 28, 28).astype(np.float32)
